# revision 1
# baseline (speedup 1.0000x reference)
"""Single-head causal attention on 8 trn2 NeuronCores (one batch element per core).

Problem: x [8, 2048, 1024], Wq/Wk/Wv [1024, 64] -> out [8, 2048, 64]
  q = x@Wq; k = x@Wk; v = x@Wv; out = causal_softmax(q k^T / sqrt(64)) @ v

Strategy (per core, batch-parallel across the 8 cores):
  - Host pre-transposes each core's x to x^T [E, S] so the QKV projections can
    contract over E with E on SBUF partitions (no on-chip transpose of x).
  - Projections on the PE as fp32r matmuls: Q^T and K^T are produced
    *duplicated* across partition halves (lhsT = [Wq|Wq]) so the score
    matmuls can be packed two-at-a-time into disjoint PE row groups.
  - Scores are computed transposed (P^T[kv, q]) so that softmax normalization
    can ride the PV matmul: V is augmented with a ones column, making row 64
    of the PV output the softmax denominator. No max-subtraction is needed
    (scores are O(1) by construction; exp cannot overflow fp32).
  - exp on ScalarE straight out of PSUM with the 1/sqrt(D) scale folded in.
  - Causal masking is a multiplicative 0/1 mask applied after exp, only on
    diagonal tiles, sliced from one precomputed [128, 1024] step mask.
  - PV accumulates out^T in PSUM; a PE transpose brings it back to natural
    layout where the per-query normalizer lands on the partition dim, so the
    divide is a reciprocal + per-partition tensor_scalar multiply.
"""

import numpy as np

import concourse.bass as bass
import concourse.mybir as mybir
import concourse.tile as tile
from concourse.vector_clock import ScopedClock

S = 2048  # sequence length
E = 1024  # embed dim
D = 64    # head size
B = 8     # batch == number of cores
P = 128   # SBUF partitions
SBLK = 512         # q-block / s-block width (max fp32 matmul moving dim)
EC = E // P        # 8 contraction chunks
NSB = S // SBLK    # 4 s-blocks
NJT = S // P       # 16 kv tiles

f32 = mybir.dt.float32
f32r = mybir.dt.float32r
f16 = mybir.dt.float16
MMDT = f16          # dtype of all large-matmul operands
MMNP = np.float16   # matching numpy dtype for host-side prep
AF = mybir.ActivationFunctionType

_PATCHED = False


def _patch_tile_drain():
    """The walrus build in this container rejects instructions carrying more
    than one sem wait on the Tile exit Drain. Split the waits across a chain
    of drains, one wait each."""
    global _PATCHED
    if _PATCHED:
        return
    _PATCHED = True

    def _drain_and_barrier(self, tick_clock, wait_clock):
        drain_inst = self.nc.sync.drain()
        wait_clock.add_sem_waits(
            drain_inst.ins, ScopedClock({None: tick_clock.global_clock})
        )
        ins = drain_inst.ins
        si = ins.sync_info
        if si is not None and si.on_wait is not None and len(si.on_wait) > 1:
            waits = list(si.on_wait)
            ins.sync_info = mybir.SyncInfo(
                on_wait=[waits[0]], on_update=list(si.on_update or [])
            )
            for w in waits[1:]:
                d2 = self.nc.sync.drain()
                d2.ins.sync_info = mybir.SyncInfo(on_wait=[w], on_update=[])
        self.nc.all_engine_barrier()
        assert self.sems is not None
        popped = self.nc._tile_sem_poison_stack.pop()
        assert popped is self._sem_poison
        self.nc.clear_and_free_semaphores(list(self.sems.allocated().values()))
        self.nc.all_engine_barrier()

    tile.TileContext._drain_and_barrier = _drain_and_barrier


def _split_multiwaits(nc):
    """This container's walrus rejects instructions carrying more than one
    sem wait (setupSyncWait: 'Too many sync wait commands'). Hoist all but
    the last wait of every instruction onto same-engine NoOps placed
    immediately before it — the engine sequencer processes them in order,
    which is semantically identical."""
    ctr = 0
    for f in nc.m.functions:
        for bb in f.blocks:
            out = []
            changed = False
            for inst in bb.instructions:
                si = inst.sync_info
                if si is not None and si.on_wait is not None and len(si.on_wait) > 1:
                    waits = list(si.on_wait)
                    for w in waits[:-1]:
                        nop = mybir.InstNoOp(name=f"I-waitsplit-{ctr}")
                        ctr += 1
                        nop.engine = inst.engine
                        nop.sync_info = mybir.SyncInfo(on_wait=[w], on_update=[])
                        out.append(nop)
                    inst.sync_info = mybir.SyncInfo(
                        on_wait=[waits[-1]], on_update=list(si.on_update or [])
                    )
                    changed = True
                out.append(inst)
            if changed:
                bb.instructions = out


def _attention(ctx, tc, xt, wqk, wv, y):
    nc = tc.nc
    scale = 1.0 / np.sqrt(D)

    persist = ctx.enter_context(tc.tile_pool(name="persist", bufs=1))
    xpool = ctx.enter_context(tc.tile_pool(name="xts", bufs=1))
    ppool = ctx.enter_context(tc.tile_pool(name="pp", bufs=6))
    opool = ctx.enter_context(tc.tile_pool(name="ot", bufs=2))
    rpool = ctx.enter_context(tc.tile_pool(name="rec", bufs=8))
    psproj = ctx.enter_context(tc.tile_pool(name="psproj", bufs=2, space="PSUM"))
    psscore = ctx.enter_context(tc.tile_pool(name="psscore", bufs=2, space="PSUM"))
    pspv = ctx.enter_context(tc.tile_pool(name="pspv", bufs=1, space="PSUM"))
    pstr = ctx.enter_context(tc.tile_pool(name="pstr", bufs=1, space="PSUM"))

    # ---- weights (dual queue: wqk on sync, wv on scalar) ----------------
    wqk_sb = persist.tile([P, EC, 2 * D], MMDT, tag="wqk")  # [Wq|Wk] packed
    wv_sb = persist.tile([P, EC, D], MMDT, tag="wv")
    nc.sync.dma_start(wqk_sb[:], wqk.rearrange("(c p) m -> p c m", p=P))
    nc.sync.dma_start(wv_sb[:], wv.rearrange("(c p) m -> p c m", p=P))

    # ---- PE warm-up: keep HAM busy while the input streams in -----------
    warm_in = persist.tile([P, SBLK], MMDT, tag="warm")
    nc.vector.memset(warm_in[:], 0.25)
    wt = pstr.tile([P, SBLK], f32, tag="tr")
    for _ in range(32):
        nc.tensor.matmul(wt[:], warm_in[:, :P], warm_in[:], start=True, stop=True)

    # ---- constants -------------------------------------------------------
    ident = persist.tile([P, P], f32, tag="ident")
    nc.gpsimd.memset(ident[:], 0.0)
    nc.gpsimd.affine_select(
        out=ident[:], in_=ident[:],
        compare_op=mybir.AluOpType.not_equal, fill=1.0,
        base=0, pattern=[[-1, P]], channel_multiplier=1,
    )
    ident16 = persist.tile([P, P], MMDT, tag="ident16")
    nc.vector.tensor_copy(ident16[:], ident[:])

    # causal step mask: maskW[jj, c] = 1 iff c >= jj + SBLK
    maskW = persist.tile([P, 2 * SBLK], f32, tag="maskw")
    nc.gpsimd.memset(maskW[:], 1.0)
    nc.gpsimd.affine_select(
        out=maskW[:], in_=maskW[:],
        compare_op=mybir.AluOpType.is_ge, fill=0.0,
        base=-SBLK, pattern=[[1, 2 * SBLK]], channel_multiplier=-1,
    )
    mask16 = persist.tile([P, 2 * SBLK], MMDT, tag="mask16")
    nc.vector.tensor_copy(mask16[:], maskW[:])

    # ---- persistent activations -----------------------------------------
    # qk: rows 0:64 = Q^T, rows 64:128 = K^T (straight from packed psum)
    qk = persist.tile([P, S], MMDT, tag="qk")
    # partition-shifted copies (SBUF->SBUF DMA): K^T at rows 0:64, Q^T at 64:128
    kTlo = persist.tile([D, S], MMDT, tag="ktlo")
    qThi = persist.tile([P, S], MMDT, tag="qthi")  # rows 64:128 used
    vT = persist.tile([D, S], MMDT, tag="vt")
    vAug = persist.tile([P, NJT, 2 * D], MMDT, tag="vaug")
    yT = persist.tile([D, S], f32, tag="ytout")
    ones_f32 = persist.tile([P, NJT, D], f32, tag="ones")
    nc.vector.memset(ones_f32[:], 1.0)
    nc.vector.tensor_copy(vAug[:, :, D:], ones_f32[:])

    # ---- stream x^T (alternate HWDGE queues) ----------------------------
    xt_r = xt.rearrange("(c p) s -> p c s", p=P)
    xts0 = xpool.tile([P, EC, SBLK], MMDT, tag="xts0")
    for c in range(0, EC, 2):
        e2 = nc.sync if (c // 2) % 2 == 0 else nc.scalar
        e2.dma_start(xts0[:, c : c + 2, :], xt_r[:, c : c + 2, :SBLK])
    # blocks 1+2 as one load: 2KB-contiguous descriptors, better line rate
    xts12 = xpool.tile([P, EC, 2 * SBLK], MMDT, tag="xts12")
    nc.scalar.dma_start(xts12[:], xt_r[:, :, SBLK : 3 * SBLK])
    xts3 = xpool.tile([P, EC, SBLK], MMDT, tag="xts3")
    nc.sync.dma_start(xts3[:], xt_r[:, :, 3 * SBLK :])
    xts = [xts0, xts12[:, :, :SBLK], xts12[:, :, SBLK:], xts3]

    def proj_chunks(b):
        """Emit-steps for s-block b's projections; the attention loop of
        block b-1 interleaves these between its pairs so the PE always has
        independent matmuls queued behind exp-dependent ones."""
        sl = slice(b * SBLK, (b + 1) * SBLK)
        steps = []
        psQK = psproj.tile([P, SBLK], f32, tag="proj")

        def qk_mms(e0, psQK=psQK):
            for e in (e0, e0 + 1):
                nc.tensor.matmul(
                    psQK[:], wqk_sb[:, e, :], xts[b][:, e, :],
                    start=(e == 0), stop=(e == EC - 1),
                )

        for e0 in range(0, EC, 2):
            steps.append(lambda e0=e0: qk_mms(e0))

        def qk_out():
            nc.vector.tensor_copy(qk[:, sl], psQK[:])
            # partition shifts via SWDGE SBUF->SBUF DMA
            nc.gpsimd.dma_start(kTlo[:, sl], qk[D:P, sl])
            nc.gpsimd.dma_start(qThi[D:P, sl], qk[:D, sl])

        steps.append(qk_out)
        psV = psproj.tile([P, SBLK], f32, tag="proj")

        def v_mms(e0, psV=psV):
            for e in (e0, e0 + 1):
                nc.tensor.matmul(
                    psV[:D, :], wv_sb[:, e, :], xts[b][:, e, :],
                    start=(e == 0), stop=(e == EC - 1),
                )

        for e0 in range(0, EC, 2):
            steps.append(lambda e0=e0: v_mms(e0))

        def v_out():
            nc.vector.tensor_copy(vT[:, sl], psV[:D, :])

        steps.append(v_out)

        def v_tr(t):
            j = 4 * b + t
            psv_t = pstr.tile([P, SBLK], MMDT, tag="tr")
            nc.tensor.transpose(
                psv_t[:, :D], vT[:, j * P : (j + 1) * P], ident16[:D, :D]
            )
            nc.vector.tensor_copy(vAug[:, j, :D], psv_t[:, :D])

        for t in range(4):
            steps.append(lambda t=t: v_tr(t))
        return steps

    def proj(b):
        for step in proj_chunks(b):
            step()

    def attn(b, bg=()):
        bg = list(bg)
        sl = slice(b * SBLK, (b + 1) * SBLK)
        nj = 4 * b + 4
        psO = pspv.tile([P, SBLK], f32, tag="pv")
        pairs = [(jp, jp + 1) for jp in range(0, nj, 2)]

        def scores_pair(pi):
            j0, j1 = pairs[pi]
            ps = psscore.tile([P, 2 * SBLK], f32, tag="score")
            # narrow only the strongly-masked tiles (t>=2); the (0,1) pair
            # stays full-width so one exp can cover both banks contiguously
            o0 = max(0, (j0 - 4 * b) * P)
            o1 = max(0, (j1 - 4 * b) * P)
            o0 = o0 if o0 >= 2 * P else 0
            o1 = o1 if o1 >= 2 * P else 0
            q0 = slice(b * SBLK + o0, (b + 1) * SBLK)
            q1 = slice(b * SBLK + o1, (b + 1) * SBLK)
            # two PE row-groups: rows 0:64 (kTlo/qk) and 64:128 (qk/qThi)
            nc.tensor.matmul(
                ps[:, o0:SBLK], kTlo[:, j0 * P : (j0 + 1) * P], qk[:D, q0],
            )
            nc.tensor.matmul(
                ps[:, SBLK + o1 :], qk[D:P, j1 * P : (j1 + 1) * P], qThi[D:P, q1],
            )
            return (j0, j1, ps)

        inflight = scores_pair(0)
        for pi in range(len(pairs)):
            j0, j1, ps = inflight
            pt = ppool.tile([P, 2 * SBLK], MMDT, tag="pt")
            offs = [max(0, (j - 4 * b) * P) for j in (j0, j1)]
            eoffs = [o if o >= 2 * P else 0 for o in offs]
            if eoffs == [0, 0]:
                # (nearly) fully-visible pair: one batched exp over both banks
                nc.scalar.activation(pt[:], ps[:], AF.Exp, scale=float(scale))
            else:
                # strongly-masked pair: exp only the causally-reachable columns
                for k, off in enumerate(eoffs):
                    nc.scalar.activation(
                        pt[:, k * SBLK + off : (k + 1) * SBLK],
                        ps[:, k * SBLK + off : (k + 1) * SBLK],
                        AF.Exp, scale=float(scale),
                    )
            for k, j in enumerate((j0, j1)):
                t = j - 4 * b
                if t >= 0:
                    off = eoffs[k]
                    nc.vector.tensor_mul(
                        pt[:, k * SBLK + off : (k + 1) * SBLK],
                        pt[:, k * SBLK + off : (k + 1) * SBLK],
                        mask16[:, SBLK - t * P + off : 2 * SBLK - t * P],
                    )
            if pi + 1 < len(pairs):
                inflight = scores_pair(pi + 1)
            # independent next-block projection work keeps the PE busy
            # while this pair's exp runs on ScalarE
            take = max(1, (len(bg) + len(pairs) - pi - 1) // max(1, len(pairs) - pi))
            for _ in range(take):
                if bg:
                    bg.pop(0)()
            for k, j in enumerate((j0, j1)):
                off = eoffs[k]
                nc.tensor.matmul(
                    psO[:, off:], vAug[:, j, :],
                    pt[:, k * SBLK + off : (k + 1) * SBLK],
                    start=(j == 0), stop=(j == nj - 1),
                )
        for step in bg:
            step()
        return psO

    def out(b, psO):
        # rows 64:128 of psO hold the softmax denominator, pre-broadcast.
        # 1/s as exp(-ln s) on ScalarE: same ACT table set as the softmax exp,
        # and ~3x faster than DVE reciprocal at this shape. Two column-halves
        # pipeline ACT -> DVE -> DMA and release the PV bank sooner.
        sl = slice(b * SBLK, (b + 1) * SBLK)
        lns = rpool.tile([D, SBLK], f32, tag="lns")
        nc.scalar.activation(lns[:], psO[D:P, :], AF.Ln)
        rcp = rpool.tile([D, SBLK], f32, tag="rcp")
        nc.scalar.activation(rcp[:], lns[:], AF.Exp, scale=-1.0)
        nc.vector.tensor_mul(yT[:, sl], psO[:D, :], rcp[:])
        if b == NSB - 1:
            # tail-critical store: halves on both HWDGE rings overlap receipts
            h0 = slice(b * SBLK, b * SBLK + SBLK // 2)
            h1 = slice(b * SBLK + SBLK // 2, (b + 1) * SBLK)
            nc.sync.dma_start(y[:, h0], yT[:, h0])
            nc.scalar.dma_start(y[:, h1], yT[:, h1])
        else:
            nc.sync.dma_start(y[:, sl], yT[:, sl])

    prev = None
    for b in range(NSB):
        proj(b)
        if prev is not None:
            out(b - 1, prev)
        prev = attn(b)
    out(NSB - 1, prev)


def build_nc():
    from contextlib import ExitStack

    _patch_tile_drain()
    nc = bass.Bass(target_bir_lowering=False, enable_partition_id=False)
    xt = nc.dram_tensor("xt", [E, S], MMDT, kind="ExternalInput")
    wqk = nc.dram_tensor("wqk", [E, 2 * D], MMDT, kind="ExternalInput")
    wv = nc.dram_tensor("wv", [E, D], MMDT, kind="ExternalInput")
    y = nc.dram_tensor("y", [D, S], f32, kind="ExternalOutput")
    with tile.TileContext(nc) as tc:
        with ExitStack() as ctx:
            _attention(ctx, tc, xt, wqk, wv, y)
    return nc


def make_in_maps(x, Wq, Wk, Wv):
    wqk = np.ascontiguousarray(np.concatenate([Wq, Wk], axis=1), dtype=MMNP)
    wv_c = np.ascontiguousarray(Wv, dtype=MMNP)
    x = np.asarray(x)
    return [
        {
            "xt": np.ascontiguousarray(x[b].T.astype(MMNP)),
            "wqk": wqk,
            "wv": wv_c,
        }
        for b in range(B)
    ]


_NC = None


def kernel(x, Wq, Wk, Wv, _trace=False, _tmpdir=None):
    from concourse.bass_utils import run_bass_kernel_spmd

    global _NC
    if _NC is None:
        _NC = build_nc()
        _split_multiwaits(_NC)  # walrus-only legalization; breaks CoreSim
    in_maps = make_in_maps(x, Wq, Wk, Wv)
    res = run_bass_kernel_spmd(
        _NC, in_maps, core_ids=list(range(B)), trace=_trace, tmpdir=_tmpdir
    )
    out = np.ascontiguousarray(
        np.stack([r["y"].T for r in res.results], axis=0), dtype=np.float32
    )
    if _trace:
        kernel.last_results = res
    return out



# revision 4
# speedup vs baseline: 1.0146x; 1.0146x over previous
"""Single-head causal attention on 8 trn2 NeuronCores (one batch element per core).

Problem: x [8, 2048, 1024], Wq/Wk/Wv [1024, 64] -> out [8, 2048, 64]
  q = x@Wq; k = x@Wk; v = x@Wv; out = causal_softmax(q k^T / sqrt(64)) @ v

Strategy (per core, batch-parallel across the 8 cores):
  - Host pre-transposes + pre-tiles each core's x to [P, NSB, EC, SBLK] fp16 so
    every DMA piece is 4KB-contiguous per partition and the QKV projections
    contract over E with E on SBUF partitions (no on-chip transpose of x).
  - Input DMAs are hoisted to the very front of the program (before the entry
    barrier) so the x stream overlaps the fixed engine-start/preamble cost.
  - Projections on the PE: Q^T and K^T packed as one [Wq|Wk] matmul; the
    chunk loop consumes the scalar-ring half (chunks 4-7) first since it
    lands in SBUF earlier.
  - Scores are computed transposed (P^T[kv, q]) two tiles at a time in
    disjoint PE row groups; softmax normalization rides the PV matmul via a
    ones column in V (row 64 of PV = denominator). No max-subtraction needed
    (scores are O(1) by construction; exp cannot overflow fp32).
  - exp on ScalarE straight out of PSUM with the 1/sqrt(D) scale folded in.
  - Causal masking is a multiplicative 0/1 mask applied after exp, only on
    diagonal tiles, sliced from one precomputed [128, 1024] step mask.
  - PV accumulates out^T in PSUM; the denominator lands on rows 64:128, so
    the divide is Ln/Exp on ScalarE + per-partition multiply on DVE.
"""

import numpy as np

import concourse.bass as bass
import concourse.mybir as mybir
import concourse.tile as tile
from concourse.vector_clock import ScopedClock

S = 2048  # sequence length
E = 1024  # embed dim
D = 64    # head size
B = 8     # batch == number of cores
P = 128   # SBUF partitions
SBLK = 512         # q-block / s-block width (max fp32 matmul moving dim)
EC = E // P        # 8 contraction chunks
NSB = S // SBLK    # 4 s-blocks
NJT = S // P       # 16 kv tiles

f32 = mybir.dt.float32
f32r = mybir.dt.float32r
f16 = mybir.dt.float16
MMDT = f16          # dtype of all large-matmul operands
MMNP = np.float16   # matching numpy dtype for host-side prep
AF = mybir.ActivationFunctionType

_PATCHED = False


def _patch_tile_drain():
    """The walrus build in this container rejects instructions carrying more
    than one sem wait on the Tile exit Drain. Split the waits across a chain
    of drains, one wait each."""
    global _PATCHED
    if _PATCHED:
        return
    _PATCHED = True

    def _drain_and_barrier(self, tick_clock, wait_clock):
        drain_inst = self.nc.sync.drain()
        wait_clock.add_sem_waits(
            drain_inst.ins, ScopedClock({None: tick_clock.global_clock})
        )
        ins = drain_inst.ins
        si = ins.sync_info
        if si is not None and si.on_wait is not None and len(si.on_wait) > 1:
            waits = list(si.on_wait)
            ins.sync_info = mybir.SyncInfo(
                on_wait=[waits[0]], on_update=list(si.on_update or [])
            )
            for w in waits[1:]:
                d2 = self.nc.sync.drain()
                d2.ins.sync_info = mybir.SyncInfo(on_wait=[w], on_update=[])
        self.nc.all_engine_barrier()
        assert self.sems is not None
        popped = self.nc._tile_sem_poison_stack.pop()
        assert popped is self._sem_poison
        self.nc.clear_and_free_semaphores(list(self.sems.allocated().values()))
        self.nc.all_engine_barrier()

    tile.TileContext._drain_and_barrier = _drain_and_barrier


def _split_multiwaits(nc):
    """This container's walrus rejects instructions carrying more than one
    sem wait (setupSyncWait: 'Too many sync wait commands'). Hoist all but
    the last wait of every instruction onto same-engine NoOps placed
    immediately before it — the engine sequencer processes them in order,
    which is semantically identical."""
    ctr = 0
    for f in nc.m.functions:
        for bb in f.blocks:
            out = []
            changed = False
            for inst in bb.instructions:
                si = inst.sync_info
                if si is not None and si.on_wait is not None and len(si.on_wait) > 1:
                    waits = list(si.on_wait)
                    for w in waits[:-1]:
                        nop = mybir.InstNoOp(name=f"I-waitsplit-{ctr}")
                        ctr += 1
                        nop.engine = inst.engine
                        nop.sync_info = mybir.SyncInfo(on_wait=[w], on_update=[])
                        out.append(nop)
                    inst.sync_info = mybir.SyncInfo(
                        on_wait=[waits[-1]], on_update=list(si.on_update or [])
                    )
                    changed = True
                out.append(inst)
            if changed:
                bb.instructions = out


def _restructure(nc):
    """Post-build program surgery:
    1. Hoist the input-load DMA instructions (collected in nc._hoist_dma) from
       the tile block to the very front of the main block, ahead of the entry
       barrier — the DMA engines run independently of the compute engines'
       barrier, so the x stream starts ~7us earlier.
    2. Move the const-pool memsets (Pool engine) from before the entry barrier
       into the tile block — their only consumers (ACT bias reads) run much
       later, and Pool arriving at the barrier earlier unblocks everyone.
    3. Shrink the dynamic DMA queue rings: the compiler's NEFF epilogue emits
       one serial semaphore-clear per allocated queue (~115ns each), so 48
       queues cost ~5.5us of pure teardown."""
    f = nc.m.functions[0]
    main, tileb = f.blocks[0], f.blocks[1]
    hoist_ids = {id(i) for i in nc._hoist_dma}
    moved = [i for i in tileb.instructions if id(i) in hoist_ids]
    assert len(moved) == len(nc._hoist_dma), (len(moved), len(nc._hoist_dma))
    # The 5th+ DMA per ring carries tile's in-flight throttle (wait for that
    # ring's 1st DMA completion sem) — safe hoisted, the sem is updated by an
    # earlier hoisted DMA on the same engine.
    rest_tile = [i for i in tileb.instructions if id(i) not in hoist_ids]
    memsets = [i for i in main.instructions if isinstance(i, mybir.InstMemset)]
    rest_main = [i for i in main.instructions if not isinstance(i, mybir.InstMemset)]
    main.instructions = rest_main[:1] + moved + rest_main[1:]
    tileb.instructions = memsets + rest_tile

    for q in nc.m.queues:
        if q.name.startswith("qPoolDynamic"):
            q.num_queues = 2
        else:
            q.num_queues = 8


def _attention(ctx, tc, xt, wqk, wv, y):
    nc = tc.nc
    scale = 1.0 / np.sqrt(D)

    persist = ctx.enter_context(tc.tile_pool(name="persist", bufs=1))
    xpool = ctx.enter_context(tc.tile_pool(name="xts", bufs=1))
    ppool = ctx.enter_context(tc.tile_pool(name="pp", bufs=6))
    rpool = ctx.enter_context(tc.tile_pool(name="rec", bufs=8))
    psproj = ctx.enter_context(tc.tile_pool(name="psproj", bufs=2, space="PSUM"))
    psscore = ctx.enter_context(tc.tile_pool(name="psscore", bufs=2, space="PSUM"))
    pspv = ctx.enter_context(tc.tile_pool(name="pspv", bufs=1, space="PSUM"))
    pstr = ctx.enter_context(tc.tile_pool(name="pstr", bufs=1, space="PSUM"))

    hoist = []

    # ---- weights + x stream, all hoisted to program start ----------------
    # sync ring: wqk, then chunk-halves 0:4 of each s-block (FIFO per ring).
    # scalar ring: wv, then chunk-halves 4:8 (these land slightly earlier, so
    # the projection chunk loops consume 4..7 first).
    wqk_sb = persist.tile([P, EC, 2 * D], MMDT, tag="wqk")
    wv_sb = persist.tile([P, EC, D], MMDT, tag="wv")
    hoist.append(nc.sync.dma_start(wqk_sb[:], wqk.rearrange("p (c m) -> p c m", c=EC)))
    hoist.append(nc.scalar.dma_start(wv_sb[:], wv.rearrange("p (c m) -> p c m", c=EC)))

    xt_r = xt.rearrange("p (b c s) -> p b c s", b=NSB, c=EC)
    xts = []
    for b in range(NSB):
        xts_b = xpool.tile([P, EC, SBLK], MMDT, tag=f"xts{b}", name=f"xts_{b}")
        xts.append(xts_b)
    EH = EC // 2
    for b in range(NSB):
        hoist.append(nc.sync.dma_start(xts[b][:, :EH, :], xt_r[:, b, :EH, :]))
    for b in range(NSB):
        hoist.append(nc.scalar.dma_start(xts[b][:, EH:, :], xt_r[:, b, EH:, :]))
    nc._hoist_dma = [h.ins for h in hoist]

    EORD = [4, 5, 6, 7, 0, 1, 2, 3]  # scalar-ring half first

    # ---- constants -------------------------------------------------------
    ident = persist.tile([P, P], f32, tag="ident")
    nc.gpsimd.memset(ident[:], 0.0)
    nc.gpsimd.affine_select(
        out=ident[:], in_=ident[:],
        compare_op=mybir.AluOpType.not_equal, fill=1.0,
        base=0, pattern=[[-1, P]], channel_multiplier=1,
    )
    ident16 = persist.tile([P, P], MMDT, tag="ident16")
    nc.vector.tensor_copy(ident16[:], ident[:])

    # causal step mask: maskW[jj, c] = 1 iff c >= jj + SBLK
    maskW = persist.tile([P, 2 * SBLK], f32, tag="maskw")
    nc.gpsimd.memset(maskW[:], 1.0)
    nc.gpsimd.affine_select(
        out=maskW[:], in_=maskW[:],
        compare_op=mybir.AluOpType.is_ge, fill=0.0,
        base=-SBLK, pattern=[[1, 2 * SBLK]], channel_multiplier=-1,
    )
    mask16 = persist.tile([P, 2 * SBLK], MMDT, tag="mask16")
    nc.vector.tensor_copy(mask16[:], maskW[:])

    # ---- persistent activations -----------------------------------------
    # qk: rows 0:64 = Q^T, rows 64:128 = K^T (straight from packed psum)
    qk = persist.tile([P, S], MMDT, tag="qk")
    # partition-shifted copies (SBUF->SBUF DMA): K^T at rows 0:64, Q^T at 64:128
    kTlo = persist.tile([D, S], MMDT, tag="ktlo")
    qThi = persist.tile([P, S], MMDT, tag="qthi")  # rows 64:128 used
    vT = persist.tile([D, S], MMDT, tag="vt")
    vAug = persist.tile([P, NJT, 2 * D], MMDT, tag="vaug")
    yT = persist.tile([D, S], f32, tag="ytout")
    ones_f32 = persist.tile([P, NJT, D], f32, tag="ones")
    nc.vector.memset(ones_f32[:], 1.0)
    nc.vector.tensor_copy(vAug[:, :, D:], ones_f32[:])

    def proj(b):
        sl = slice(b * SBLK, (b + 1) * SBLK)
        psQK = psproj.tile([P, SBLK], f32, tag="proj")
        for i, e in enumerate(EORD):
            nc.tensor.matmul(
                psQK[:], wqk_sb[:, e, :], xts[b][:, e, :],
                start=(i == 0), stop=(i == EC - 1),
            )
        nc.vector.tensor_copy(qk[:, sl], psQK[:])
        # partition shifts via SWDGE SBUF->SBUF DMA
        nc.gpsimd.dma_start(kTlo[:, sl], qk[D:P, sl])
        nc.gpsimd.dma_start(qThi[D:P, sl], qk[:D, sl])
        psV = psproj.tile([P, SBLK], f32, tag="proj")
        for i, e in enumerate(EORD):
            nc.tensor.matmul(
                psV[:D, :], wv_sb[:, e, :], xts[b][:, e, :],
                start=(i == 0), stop=(i == EC - 1),
            )
        nc.vector.tensor_copy(vT[:, sl], psV[:D, :])
        for t in range(4):
            j = 4 * b + t
            psv_t = pstr.tile([P, SBLK], MMDT, tag="tr")
            nc.tensor.transpose(
                psv_t[:, :D], vT[:, j * P : (j + 1) * P], ident16[:D, :D]
            )
            nc.vector.tensor_copy(vAug[:, j, :D], psv_t[:, :D])

    def attn(b):
        sl = slice(b * SBLK, (b + 1) * SBLK)
        nj = 4 * b + 4
        psO = pspv.tile([P, SBLK], f32, tag="pv")
        pairs = [(jp, jp + 1) for jp in range(0, nj, 2)]

        def scores_pair(pi):
            j0, j1 = pairs[pi]
            ps = psscore.tile([P, 2 * SBLK], f32, tag="score")
            # narrow only the strongly-masked tiles (t>=2); the (0,1) pair
            # stays full-width so one exp can cover both banks contiguously
            o0 = max(0, (j0 - 4 * b) * P)
            o1 = max(0, (j1 - 4 * b) * P)
            o0 = o0 if o0 >= 2 * P else 0
            o1 = o1 if o1 >= 2 * P else 0
            q0 = slice(b * SBLK + o0, (b + 1) * SBLK)
            q1 = slice(b * SBLK + o1, (b + 1) * SBLK)
            # two PE row-groups: rows 0:64 (kTlo/qk) and 64:128 (qk/qThi)
            nc.tensor.matmul(
                ps[:, o0:SBLK], kTlo[:, j0 * P : (j0 + 1) * P], qk[:D, q0],
            )
            nc.tensor.matmul(
                ps[:, SBLK + o1 :], qk[D:P, j1 * P : (j1 + 1) * P], qThi[D:P, q1],
            )
            return (j0, j1, ps)

        inflight = scores_pair(0)
        for pi in range(len(pairs)):
            j0, j1, ps = inflight
            pt = ppool.tile([P, 2 * SBLK], MMDT, tag="pt")
            offs = [max(0, (j - 4 * b) * P) for j in (j0, j1)]
            eoffs = [o if o >= 2 * P else 0 for o in offs]
            if eoffs == [0, 0]:
                # (nearly) fully-visible pair: one batched exp over both banks
                nc.scalar.activation(pt[:], ps[:], AF.Exp, scale=float(scale))
            else:
                # strongly-masked pair: exp only the causally-reachable columns
                for k, off in enumerate(eoffs):
                    nc.scalar.activation(
                        pt[:, k * SBLK + off : (k + 1) * SBLK],
                        ps[:, k * SBLK + off : (k + 1) * SBLK],
                        AF.Exp, scale=float(scale),
                    )
            for k, j in enumerate((j0, j1)):
                t = j - 4 * b
                if t >= 0:
                    off = eoffs[k]
                    nc.vector.tensor_mul(
                        pt[:, k * SBLK + off : (k + 1) * SBLK],
                        pt[:, k * SBLK + off : (k + 1) * SBLK],
                        mask16[:, SBLK - t * P + off : 2 * SBLK - t * P],
                    )
            if pi + 1 < len(pairs):
                inflight = scores_pair(pi + 1)
            for k, j in enumerate((j0, j1)):
                off = eoffs[k]
                nc.tensor.matmul(
                    psO[:, off:], vAug[:, j, :],
                    pt[:, k * SBLK + off : (k + 1) * SBLK],
                    start=(j == 0), stop=(j == nj - 1),
                )
        return psO

    def out(b, psO):
        # rows 64:128 of psO hold the softmax denominator, pre-broadcast.
        # 1/s as exp(-ln s) on ScalarE: same ACT table set as the softmax exp,
        # and ~3x faster than DVE reciprocal at this shape.
        sl = slice(b * SBLK, (b + 1) * SBLK)
        lns = rpool.tile([D, SBLK], f32, tag="lns")
        nc.scalar.activation(lns[:], psO[D:P, :], AF.Ln)
        rcp = rpool.tile([D, SBLK], f32, tag="rcp")
        nc.scalar.activation(rcp[:], lns[:], AF.Exp, scale=-1.0)
        nc.vector.tensor_mul(yT[:, sl], psO[:D, :], rcp[:])
        if b == NSB - 1:
            # tail-critical store: halves on both HWDGE rings overlap receipts
            h0 = slice(b * SBLK, b * SBLK + SBLK // 2)
            h1 = slice(b * SBLK + SBLK // 2, (b + 1) * SBLK)
            nc.sync.dma_start(y[:, h0], yT[:, h0])
            nc.scalar.dma_start(y[:, h1], yT[:, h1])
        else:
            nc.sync.dma_start(y[:, sl], yT[:, sl])

    prev = None
    for b in range(NSB):
        proj(b)
        if prev is not None:
            out(b - 1, prev)
        prev = attn(b)
    out(NSB - 1, prev)


def build_nc():
    from contextlib import ExitStack

    _patch_tile_drain()
    nc = bass.Bass(target_bir_lowering=False, enable_partition_id=False)
    xt = nc.dram_tensor("xt", [P, NSB * EC * SBLK], MMDT, kind="ExternalInput")
    wqk = nc.dram_tensor("wqk", [P, EC * 2 * D], MMDT, kind="ExternalInput")
    wv = nc.dram_tensor("wv", [P, EC * D], MMDT, kind="ExternalInput")
    y = nc.dram_tensor("y", [D, S], f32, kind="ExternalOutput")
    with tile.TileContext(nc) as tc:
        with ExitStack() as ctx:
            _attention(ctx, tc, xt, wqk, wv, y)
    _restructure(nc)
    return nc


def make_in_maps(x, Wq, Wk, Wv):
    # weights pre-tiled to [P, EC, cols]: row (c p) of W -> [p][c]
    wqk_cat = np.concatenate([Wq, Wk], axis=1).astype(MMNP)  # [E, 2D]
    wqk_arr = np.ascontiguousarray(
        wqk_cat.reshape(EC, P, 2 * D).transpose(1, 0, 2).reshape(P, EC * 2 * D)
    )
    wv_arr = np.ascontiguousarray(
        np.asarray(Wv).astype(MMNP).reshape(EC, P, D).transpose(1, 0, 2).reshape(P, EC * D)
    )
    x = np.asarray(x)
    maps = []
    for b in range(B):
        xt = x[b].T.astype(MMNP)  # [E, S]
        # [P, NSB, EC, SBLK]: xa[p, blk, c, s] = xt[c*128+p, blk*512+s]
        xa = xt.reshape(EC, P, NSB, SBLK).transpose(1, 2, 0, 3)
        maps.append(
            {
                "xt": np.ascontiguousarray(xa.reshape(P, NSB * EC * SBLK)),
                "wqk": wqk_arr,
                "wv": wv_arr,
            }
        )
    return maps


_NC = None


def kernel(x, Wq, Wk, Wv, _trace=False, _tmpdir=None):
    from concourse.bass_utils import run_bass_kernel_spmd

    global _NC
    if _NC is None:
        _NC = build_nc()
        _split_multiwaits(_NC)  # walrus-only legalization; breaks CoreSim
    in_maps = make_in_maps(x, Wq, Wk, Wv)
    res = run_bass_kernel_spmd(
        _NC, in_maps, core_ids=list(range(B)), trace=_trace, tmpdir=_tmpdir
    )
    out = np.ascontiguousarray(
        np.stack([r["y"].T for r in res.results], axis=0), dtype=np.float32
    )
    if _trace:
        kernel.last_results = res
    return out
